# revision 2
# baseline (speedup 1.0000x reference)
"""MoE MLP (dense all-expert, top-2 routing weights) on 8 TRN2 NeuronCores.

Strategy: data-parallel over tokens. Each core takes N/8 = 512 tokens and
computes all 8 experts for them with fp32r (TF32-like) matmuls, weighting
expert outputs by on-device-computed top-2 routing weights. Outputs are
disjoint token slices -> no collectives.

Self-contained: hardcodes shapes from the problem spec.
"""

import os
import numpy as np

B, T, H, D, E = 2, 2048, 1024, 1024, 8
N = B * T            # 4096 tokens
NCORES = 8
TPC = N // NCORES    # 512 tokens per core
KT = H // 128        # 8 contraction tiles for mm1 / router
QT = 2 * D // 128    # 16 column tiles of gate_up (first 8 = gate, last 8 = up)
DT = D // 128        # 8 contraction tiles for mm2
TT = TPC // 128      # 4 token tiles per core
NB = H // 512        # 2 output free-dim blocks

LAST_EXEC_NS = None

_CACHE = {}


def _build_nc():
    import concourse.mybir as mybir
    import concourse.tile as tile
    from concourse import bacc

    f32 = mybir.dt.float32
    f32r = mybir.dt.float32r
    AF = mybir.ActivationFunctionType
    OP = mybir.AluOpType
    AX = mybir.AxisListType

    nc = bacc.Bacc("TRN2", target_bir_lowering=False, debug=False,
                   num_devices=NCORES)

    xT = nc.dram_tensor("xT", [H, TPC], f32r, kind="ExternalInput").ap()
    gwT = nc.dram_tensor("gwT", [H, E], f32r, kind="ExternalInput").ap()
    # w1[e, qt, p, kt, q] = gate_up_proj[e, kt*128+p, qt*128+q]
    w1 = nc.dram_tensor("w1", [E, QT, 128, KT, 128], f32r,
                        kind="ExternalInput").ap()
    w2 = nc.dram_tensor("w2", [E, D, H], f32r, kind="ExternalInput").ap()
    out = nc.dram_tensor("out", [TPC, H], f32, kind="ExternalOutput").ap()

    with tile.TileContext(nc) as tc:
        with (
            tc.tile_pool(name="persist", bufs=1) as persist,
            tc.tile_pool(name="w1p", bufs=4) as w1p,
            tc.tile_pool(name="w2p", bufs=2) as w2p,
            tc.tile_pool(name="hp", bufs=2) as hp,
            tc.tile_pool(name="tmp", bufs=3) as tmp,
            tc.tile_pool(name="rt", bufs=2) as rt,
            tc.tile_pool(name="psgu", bufs=2, space="PSUM") as psgu,
            tc.tile_pool(name="pso", bufs=3, space="PSUM") as pso,
        ):
            # ---- resident tiles ----
            xts = persist.tile([128, KT, TPC], f32r)   # x shard, [h_in_tile, kt, t]
            nc.sync.dma_start(out=xts,
                              in_=xT.rearrange("(kt p) t -> p kt t", p=128))
            gwts = persist.tile([128, KT, E], f32r)
            nc.sync.dma_start(out=gwts,
                              in_=gwT.rearrange("(kt p) e -> p kt e", p=128))
            wfin = persist.tile([128, TT, E], f32)    # routing weights [t, tt, e]
            acc = persist.tile([128, TT, H], f32)     # output accumulator

            # ---- router: logits -> top-2 normalized weights ----
            for tt in range(TT):
                pr = pso.tile([128, E], f32, tag="o")
                for kt in range(KT):
                    nc.tensor.matmul(
                        pr,
                        lhsT=xts[:, kt, tt * 128:(tt + 1) * 128],
                        rhs=gwts[:, kt, :],
                        start=(kt == 0), stop=(kt == KT - 1),
                    )
                logits = rt.tile([128, E], f32, tag="logits")
                nc.vector.tensor_copy(logits, pr)
                m1 = rt.tile([128, 1], f32, tag="m1")
                nc.vector.tensor_reduce(m1, logits, axis=AX.X, op=OP.max)
                nm1 = rt.tile([128, 1], f32, tag="nm1")
                nc.vector.tensor_scalar_mul(nm1, m1, -1.0)
                exps = rt.tile([128, E], f32, tag="exps")
                nc.scalar.activation(exps, logits, AF.Exp, bias=nm1, scale=1.0)
                eq1 = rt.tile([128, E], f32, tag="eq1")
                nc.vector.tensor_scalar(eq1, logits, m1, None, OP.is_ge)
                msk = rt.tile([128, E], f32, tag="msk")
                nc.vector.scalar_tensor_tensor(msk, in0=eq1, scalar=-1e30,
                                               in1=logits, op0=OP.mult,
                                               op1=OP.add)
                m2 = rt.tile([128, 1], f32, tag="m2")
                nc.vector.tensor_reduce(m2, msk, axis=AX.X, op=OP.max)
                top2 = rt.tile([128, E], f32, tag="top2")
                nc.vector.tensor_scalar(top2, logits, m2, None, OP.is_ge)
                wu = rt.tile([128, E], f32, tag="wu")
                nc.vector.tensor_mul(wu, exps, top2)
                s = rt.tile([128, 1], f32, tag="s")
                nc.vector.tensor_reduce(s, wu, axis=AX.X, op=OP.add)
                rs = rt.tile([128, 1], f32, tag="rs")
                nc.vector.reciprocal(rs, s)
                nc.vector.tensor_scalar_mul(wfin[:, tt, :], wu, rs)

            # ---- experts ----
            for e in range(E):
                h = hp.tile([128, DT, TPC], f32r, tag="h")  # [d_in_tile, dt, t]
                for dt in range(DT):
                    w1g = w1p.tile([128, KT, 128], f32r, tag="w1g")
                    nc.sync.dma_start(out=w1g, in_=w1[e, dt])
                    w1u = w1p.tile([128, KT, 128], f32r, tag="w1u")
                    nc.sync.dma_start(out=w1u, in_=w1[e, dt + DT])
                    pg = psgu.tile([128, TPC], f32, tag="g")
                    pu = psgu.tile([128, TPC], f32, tag="u")
                    for kt in range(KT):
                        nc.tensor.matmul(pg,
                                         lhsT=w1g[:, kt, :],
                                         rhs=xts[:, kt, :],
                                         start=(kt == 0), stop=(kt == KT - 1))
                    for kt in range(KT):
                        nc.tensor.matmul(pu,
                                         lhsT=w1u[:, kt, :],
                                         rhs=xts[:, kt, :],
                                         start=(kt == 0), stop=(kt == KT - 1))
                    sg = tmp.tile([128, TPC], f32, tag="sg")
                    nc.scalar.activation(sg, pg, AF.Silu)
                    nc.vector.tensor_mul(h[:, dt, :], sg, pu)

                w2t = w2p.tile([128, DT, H], f32r, tag="w2")
                for dt in range(DT):
                    nc.sync.dma_start(out=w2t[:, dt, :],
                                      in_=w2[e, dt * 128:(dt + 1) * 128, :])

                for tt in range(TT):
                    for nb in range(NB):
                        po = pso.tile([128, 512], f32, tag="o")
                        for dt in range(DT):
                            nc.tensor.matmul(
                                po,
                                lhsT=h[:, dt, tt * 128:(tt + 1) * 128],
                                rhs=w2t[:, dt, nb * 512:(nb + 1) * 512],
                                start=(dt == 0), stop=(dt == DT - 1))
                        a = acc[:, tt, nb * 512:(nb + 1) * 512]
                        wcol = wfin[:, tt, e:e + 1]
                        if e == 0:
                            nc.vector.tensor_scalar_mul(a, po, wcol)
                        else:
                            nc.vector.scalar_tensor_tensor(
                                a, in0=po, scalar=wcol, in1=a,
                                op0=OP.mult, op1=OP.add)

            # ---- store ----
            outr = out.rearrange("(tt p) hh -> p tt hh", p=128)
            for tt in range(TT):
                nc.sync.dma_start(out=outr[:, tt, :], in_=acc[:, tt, :])

    nc.compile()
    return nc


def _get_nc():
    if "nc" not in _CACHE:
        _CACHE["nc"] = _build_nc()
    return _CACHE["nc"]


def kernel(x, gate_w, gate_up_proj, down_proj):
    from concourse.bass_utils import run_bass_kernel_spmd

    global LAST_EXEC_NS

    x = np.ascontiguousarray(np.asarray(x, dtype=np.float32))
    gate_w = np.ascontiguousarray(np.asarray(gate_w, dtype=np.float32))
    gup = np.ascontiguousarray(np.asarray(gate_up_proj, dtype=np.float32))
    dwn = np.ascontiguousarray(np.asarray(down_proj, dtype=np.float32))

    hidden = x.reshape(N, H)
    gwT = np.ascontiguousarray(gate_w.T)                      # [H, E]
    # [E, QT, 128p, KT, 128q]: w1[e,qt,p,kt,q] = gup[e, kt*128+p, qt*128+q]
    w1 = np.ascontiguousarray(
        gup.reshape(E, KT, 128, QT, 128).transpose(0, 3, 2, 1, 4))

    nc = _get_nc()

    in_maps = []
    for c in range(NCORES):
        xTc = np.ascontiguousarray(hidden[c * TPC:(c + 1) * TPC].T)  # [H, TPC]
        in_maps.append({"xT": xTc, "gwT": gwT, "w1": w1, "w2": dwn})

    res = run_bass_kernel_spmd(
        nc, in_maps, core_ids=list(range(NCORES)),
        trace=bool(os.environ.get("KERNEL_TRACE")))
    LAST_EXEC_NS = res.exec_time_ns

    out = np.concatenate([res.results[c]["out"] for c in range(NCORES)],
                         axis=0)
    return out.reshape(B, T, H)
